# revision 1
# baseline (speedup 1.0000x reference)
"""AttentionAggregator Trainium2 kernel (8-core SPMD, data-parallel over nodes).

Reference computation (per node n, K=32 neighbors, D=128, H=32, O=128):
  att(x) = tanh(x @ W1) @ W2
  scores[n,k] = <att(neib[n,k]), att(node[n])>
  ws = softmax_k(scores);  agg[n] = sum_k ws[n,k] * neib[n,k]
  out = relu([node @ W_node, agg @ W_neib])

Device strategy (per core: 6272 nodes = 49 supertiles of 128 nodes; each
supertile = 4096 neighbor rows = 32 chunks of 128 rows):
  * scores fold: <u W2, v W2> = u @ (W2 W2^T) @ v^T, M2 = W2 W2^T precomputed
    on host, so the per-neighbor att2 matmul disappears:
    scores[n,k] = u[n,k] . w[n],  u = tanh(neib @ W1),  w = tanh(node@W1) @ M2
  * neib is cast to bf16 on host and loaded twice from HBM: natural layout
    [rows, D] (moving operand of the aggregation matmuls) and XBAR-transposed
    [D, rows] (stationary of the att matmul, which then emits u in natural
    [rows, H] layout so softmax/score work runs at full 128-partition width).
  * softmax runs max-free (tanh bounds |scores|) with deferred normalization:
    E = exp(scores); the aggregation matmul gets a 129th ones-column so
    Z = sum_k E arrives in the same PSUM tile; agg = agg_un * (1/Z).
  * aggregation: per chunk t (nodes 4t..4t+3) a block-diagonal stationary
    Wsel[(j,k), c] = E[node 4t+j, k] * (c == 4*(t%8)+j) against the natural
    chunk; 8 chunks accumulate a [32 nodes, 129] PSUM tile; 4 groups per
    supertile.
  * w replication across K goes through a DRAM scratch (write [128,32] once,
    read back with a k-broadcast access pattern), since cross-partition
    replication is not expressible on the compute engines.
"""

import sys

sys.path.insert(0, "/opt/trn_rl_repo")

import numpy as np
import ml_dtypes

N, K, D, H, O = 50000, 32, 128, 32, 128
NCORES = 8
ST_FULL = 49          # supertiles per core
NODES_ST = 128        # nodes per supertile
CH = 32               # 128-row chunks per supertile
RP = 128              # rows per chunk
NC_FULL = ST_FULL * NODES_ST          # 6272 nodes/core
NPAD = NC_FULL * NCORES               # 50176

_module_cache = {}


def _sel4_const():
    s = np.zeros((4, 128), dtype=ml_dtypes.bfloat16)
    for j in range(4):
        s[j, 32 * j : 32 * (j + 1)] = 1.0
    return s


def _patch_tile_drain():
    """This container's walrus rejects >1 sync-wait on one instruction; spread
    the TileContext tail-drain waits over extra sync nops."""
    from concourse import mybir
    from concourse import tile as tile_mod
    from concourse.tile import TileContext

    if getattr(TileContext, "_drain_patched", False):
        return
    MAXW = 1

    def _drain_and_barrier(self, tick_clock, wait_clock):
        drain_inst = self.nc.sync.drain()
        wait_clock.add_sem_waits(
            drain_inst.ins, tile_mod.ScopedClock({None: tick_clock.global_clock})
        )
        mi = drain_inst.ins
        ws = list(mi.sync_info.on_wait)
        if len(ws) > MAXW:
            mi.sync_info.on_wait = ws[:MAXW]
            rest = ws[MAXW:]
            for i in range(0, len(rest), MAXW):
                nop = self.nc.sync.nop(nofuse=True)
                nmi = nop.ins
                if nmi.sync_info is None:
                    nmi.sync_info = mybir.SyncInfo(
                        on_wait=rest[i : i + MAXW], on_update=[]
                    )
                else:
                    nmi.sync_info.on_wait = rest[i : i + MAXW]
        self.nc.all_engine_barrier()
        assert self.sems is not None
        popped = self.nc._tile_sem_poison_stack.pop()
        assert popped is self._sem_poison
        self.nc.clear_and_free_semaphores(list(self.sems.allocated().values()))
        self.nc.all_engine_barrier()

    TileContext._drain_and_barrier = _drain_and_barrier
    TileContext._drain_patched = True


def _split_multi_waits(nc, maxw=1):
    """Walrus in this container allows only one sync-wait per instruction:
    hoist extra waits onto same-engine NOPs inserted just before."""
    from concourse import mybir

    nsplit = 0
    for f in nc.m.functions:
        for b in f.blocks:
            changed = False
            out = []
            for inst in list(b.instructions):
                si = getattr(inst, "sync_info", None)
                ws = list(si.on_wait) if si is not None and si.on_wait else []
                if len(ws) > maxw:
                    keep = ws[-maxw:]
                    rest = ws[:-maxw]
                    for i in range(0, len(rest), maxw):
                        nop = mybir.InstNoOp(
                            name=f"I-wsplit{nc.next_id()}", ins=[], outs=[]
                        )
                        nop.engine = inst.engine
                        nop.sync_info = mybir.SyncInfo(
                            on_wait=rest[i : i + maxw], on_update=[]
                        )
                        out.append(nop)
                    si.on_wait = keep
                    changed = True
                    nsplit += 1
                out.append(inst)
            if changed:
                b.instructions = out
    return nsplit


def build_module(st=ST_FULL, ablate=(), repeat=1, bufs_bigs=3, bufs_mids=3, bufs_uw=4):
    import concourse.bass as bass
    from concourse import mybir
    from concourse.tile import TileContext
    from concourse.masks import make_identity

    ablate = set(ablate)
    _patch_tile_drain()

    f32 = mybir.dt.float32
    bf16 = mybir.dt.bfloat16
    AF = mybir.ActivationFunctionType
    ALU = mybir.AluOpType
    ncn = st * NODES_ST  # nodes this build handles per core

    nc = bass.Bass()
    node = nc.declare_dram_parameter("node", [ncn, D], f32, isOutput=False)
    neib = nc.declare_dram_parameter("neib", [st, CH, RP, D], bf16, isOutput=False)
    w1f = nc.declare_dram_parameter("w1f", [D, H], f32, isOutput=False)
    w1b = nc.declare_dram_parameter("w1b", [D, H], bf16, isOutput=False)
    m2 = nc.declare_dram_parameter("m2", [H, H], f32, isOutput=False)
    wnode = nc.declare_dram_parameter("wnode", [D, O], f32, isOutput=False)
    wneib = nc.declare_dram_parameter("wneib", [D, O], f32, isOutput=False)
    sel4p = nc.declare_dram_parameter("sel4", [4, 128], bf16, isOutput=False)
    out = nc.declare_dram_parameter("out", [ncn, 2 * O], f32, isOutput=True)
    # w scratch in DRAM: [supertile, node-in-supertile, h]
    wscr = nc.dram_tensor("wscr", [st, NODES_ST, H], bf16)

    with TileContext(nc) as tc:
        with (
            tc.tile_pool(name="singles", bufs=1) as singles,
            tc.tile_pool(name="nodep", bufs=3) as nodep,
            tc.tile_pool(name="bigs", bufs=bufs_bigs) as bigs,
            tc.tile_pool(name="mids", bufs=bufs_mids) as mids,
            tc.tile_pool(name="outs", bufs=3) as outs,
            tc.tile_pool(name="ps_uw", bufs=bufs_uw, space="PSUM") as ps_uw,
            tc.tile_pool(name="ps_agg", bufs=2, space="PSUM") as ps_agg,
            tc.tile_pool(name="ps_small", bufs=2, space="PSUM") as ps_small,
        ):
            # ---- one-time constants ----
            ident128 = singles.tile([128, 128], f32)
            make_identity(nc, ident128)
            ident32 = singles.tile([32, 32], f32)
            make_identity(nc, ident32)
            w1f_sb = singles.tile([D, H], f32)
            nc.gpsimd.dma_start(out=w1f_sb, in_=w1f[:, :])
            w1b_sb = singles.tile([D, H], bf16)
            nc.gpsimd.dma_start(out=w1b_sb, in_=w1b[:, :])
            m2_sb = singles.tile([H, H], f32)
            nc.gpsimd.dma_start(out=m2_sb, in_=m2[:, :])
            wnode_sb = singles.tile([D, O], f32)
            nc.gpsimd.dma_start(out=wnode_sb, in_=wnode[:, :])
            wneib_sb = singles.tile([D, O], f32)
            nc.gpsimd.dma_start(out=wneib_sb, in_=wneib[:, :])
            # maskW[p, tm, c] = 1 if c == 4*tm + p//32 else 0   (bf16)
            maskw = singles.tile([128, 8, 32], bf16)
            nc.vector.memset(maskw, 0.0)
            for tm in range(8):
                for j in range(4):
                    nc.vector.memset(
                        maskw[32 * j : 32 * j + 32, tm : tm + 1, 4 * tm + j : 4 * tm + j + 1],
                        1.0,
                    )
            ones1 = singles.tile([128, 1], bf16)
            nc.vector.memset(ones1, 1.0)
            # sel4[j, p] = 1 if p//32 == j (partition-broadcast selector)
            sel4 = singles.tile([4, 128], bf16)
            nc.gpsimd.dma_start(out=sel4, in_=sel4p[:, :])

            out_tiles = {}

            def node_path(s):
                """node tile s: out[:,0:128], and w[n]=tanh(node@W1)@M2 -> wscr[s]."""
                node_sb = nodep.tile([128, D], f32, tag="node_sb")
                nc.scalar.dma_start(out=node_sb, in_=node[s * 128 : (s + 1) * 128, :])
                nodeT_ps = ps_small.tile([128, 128], f32, tag="small")
                nc.tensor.transpose(nodeT_ps, node_sb, ident128)
                nodeT_sb = nodep.tile([128, 128], f32, tag="nodeT_sb")
                nc.scalar.copy(nodeT_sb, nodeT_ps)
                # out1 = relu(node @ W_node)
                out1_ps = ps_small.tile([128, O], f32, tag="small")
                nc.tensor.matmul(out1_ps, lhsT=nodeT_sb, rhs=wnode_sb)
                out_sb = outs.tile([128, 2 * O], f32, tag="out_sb")
                out_tiles[s] = out_sb
                nc.scalar.activation(out_sb[:, 0:O], out1_ps, AF.Relu)
                # vT = tanh(W1^T @ nodeT) : [H, 128]
                vT_ps = ps_small.tile([H, 128], f32, tag="small")
                nc.tensor.matmul(vT_ps, lhsT=w1f_sb, rhs=nodeT_sb)
                vT_sb = nodep.tile([H, 128], f32, tag="vT_sb")
                nc.scalar.activation(vT_sb, vT_ps, AF.Tanh)
                # w = v @ M2 : [128, H]
                w_ps = ps_small.tile([128, H], f32, tag="small")
                nc.tensor.matmul(w_ps, lhsT=vT_sb, rhs=m2_sb)
                w_sb = nodep.tile([128, H], bf16, tag="w_sb")
                nc.scalar.copy(w_sb, w_ps)
                nc.sync.dma_start(out=wscr[s : s + 1, :, :], in_=w_sb)

            def main_path(s):
                """neighbor attention + aggregation for supertile s."""
                # natural bf16 load: nb[p, t, 0:128] = neib row (s,t,p); col 128 = 1.0
                nb = bigs.tile([128, CH, 132], bf16, tag="nb")
                if "nb" not in ablate:
                    nc.scalar.dma_start(
                        out=nb[:, :, 0:D],
                        in_=neib[s : s + 1, :, :, :].rearrange("o t p d -> p (o t) d"),
                    )
                nc.vector.memset(nb[:, :, D : D + 1], 1.0)
                # XBAR-transposed load: nbT[d, 128*t + p]
                nbT = bigs.tile([128, CH * RP], bf16, tag="nbT")
                if "noxbar" in ablate:
                    nc.vector.memset(nbT[:, 0:4], 0.0)
                elif "xbar" in ablate:
                    nc.sync.dma_start(
                        out=nbT,
                        in_=neib[s : s + 1, :, :, :].rearrange("o t p d -> p (o t) d"),
                    )
                else:
                    nc.sync.dma_start(
                        out=nbT,
                        in_=neib[s : s + 1, :, :, :].rearrange("o t p d -> (o t p) d"),
                        transpose=True,
                    )
                # u = tanh(neib @ W1) in natural layout, chunk by chunk.
                u_sb = mids.tile([128, CH, H], bf16, tag="u")
                if "umm" in ablate:
                    nc.vector.memset(u_sb[:, 0:1, :], 0.0)
                for half in range(2) if "umm" not in ablate else ():
                    u_ps = ps_uw.tile([128, 16 * H], f32, tag="uw")
                    for tt in range(16):
                        t = 16 * half + tt
                        nc.tensor.matmul(
                            u_ps[:, tt * H : (tt + 1) * H],
                            lhsT=nbT[:, t * RP : (t + 1) * RP],
                            rhs=w1b_sb,
                        )
                    nc.scalar.activation(
                        u_sb[:, 16 * half : 16 * (half + 1), :],
                        u_ps[:, :].rearrange("p (t h) -> p t h", h=H),
                        AF.Tanh,
                    )
                # w replicated over k: wrep[32j+k, t, h] = w[4t+j, h].
                # Tiny DRAM load w4[j, t, h] = w[4t+j, h], then PE broadcast
                # via sel4 (out[p] = w4[p//32]).
                wrep = mids.tile([128, CH, H], bf16, tag="wrep")
                if "wrep" in ablate:
                    nc.vector.memset(wrep[:, 0:1, :], 0.0)
                else:
                    w4 = mids.tile([4, CH, H], bf16, tag="w4")
                    base = wscr[s : s + 1, 0:1, 0:1]
                    in_ap = bass.AP(
                        tensor=base.tensor,
                        offset=base.offset,
                        ap=[[H, 4], [4 * H, CH], [1, H]],
                    )
                    nc.sync.dma_start(out=w4, in_=in_ap)
                    w4f = w4[:, :, :].rearrange("j t h -> j (t h)")
                    for hh in range(2):
                        wrep_ps = ps_uw.tile([128, 512], f32, tag="uw")
                        nc.tensor.matmul(
                            wrep_ps, lhsT=sel4, rhs=w4f[:, 512 * hh : 512 * (hh + 1)]
                        )
                        nc.scalar.copy(
                            wrep[:, 16 * hh : 16 * (hh + 1), :],
                            wrep_ps[:, :].rearrange("p (t h) -> p t h", h=H),
                        )
                # scores[p, t] = sum_h u[p,t,h] * wrep[p,t,h]
                tmp = mids.tile([128, CH, H], bf16, tag="tmp")
                nc.vector.tensor_mul(tmp, u_sb, wrep)
                scores = mids.tile([128, CH], f32, tag="scores")
                nc.vector.tensor_reduce(
                    scores, tmp, axis=mybir.AxisListType.X, op=ALU.add
                )
                e_sb = mids.tile([128, CH], bf16, tag="e")
                nc.scalar.activation(e_sb, scores, AF.Exp)
                # wsel[p, (g,tm), c] = E[p, (g,tm)] * maskW[p, tm, c]
                wsel = mids.tile([128, CH, 32], bf16, tag="wsel")
                e_ap = e_sb[:, :]
                e_b = bass.AP(
                    tensor=e_ap.tensor,
                    offset=e_ap.offset,
                    ap=[e_ap.ap[0], [8 * e_ap.ap[1][0], 4], [e_ap.ap[1][0], 8], [0, 32]],
                )
                m_ap = maskw[:, :, :]
                m_b = bass.AP(
                    tensor=m_ap.tensor,
                    offset=m_ap.offset,
                    ap=[m_ap.ap[0], [0, 4], m_ap.ap[1], m_ap.ap[2]],
                )
                wsel_v = wsel[:, :, :].rearrange("p (g tm) c -> p g tm c", g=4)
                nc.vector.tensor_tensor(wsel_v, e_b, m_b, op=ALU.mult)
                # aggregation: 4 groups of 8 chunks -> [32 nodes, 129] PSUM,
                # then normalize by the ones-column sum and transpose into aggT.
                aggT_ps = ps_small.tile([128, 128], f32, tag="small")
                for g in range(4):
                    agg_ps = ps_agg.tile([32, 132], f32, tag="agg")
                    if "agg" in ablate:
                        nc.vector.memset(agg_ps[:, 0:132], 0.0)
                    for tm in range(8) if "agg" not in ablate else ():
                        t = 8 * g + tm
                        nc.tensor.matmul(
                            agg_ps[:, 0 : D + 1],
                            lhsT=wsel[:, t : t + 1, :],
                            rhs=nb[:, t : t + 1, 0 : D + 1],
                            start=(tm == 0),
                            stop=(tm == 7),
                        )
                    rz = mids.tile([32, 1], f32, tag="rz")
                    nc.vector.reciprocal(rz, agg_ps[:, D : D + 1])
                    agg_sb = mids.tile([32, D], f32, tag="agg_sb")
                    nc.vector.tensor_scalar(
                        agg_sb, agg_ps[:, 0:D], rz, None, op0=ALU.mult
                    )
                    nc.tensor.transpose(
                        aggT_ps[:, 32 * g : 32 * (g + 1)], agg_sb, ident32
                    )
                aggT_sb = mids.tile([128, 128], f32, tag="aggT_sb")
                nc.scalar.copy(aggT_sb, aggT_ps)
                out2_ps = ps_small.tile([128, O], f32, tag="small")
                nc.tensor.matmul(out2_ps, lhsT=aggT_sb, rhs=wneib_sb)
                out_sb = out_tiles.pop(s, None)
                if out_sb is None:  # node path ablated
                    out_sb = outs.tile([128, 2 * O], f32, tag="out_sb")
                nc.scalar.activation(out_sb[:, O : 2 * O], out2_ps, AF.Relu)
                nc.scalar.dma_start(
                    out=out[s * 128 : (s + 1) * 128, :], in_=out_sb
                )

            for _rep in range(repeat):
                for i in range(st + 1):
                    if i < st and "nodep" not in ablate:
                        node_path(i)
                    if i >= 1:
                        main_path(i - 1)

    _split_multi_waits(nc)
    return nc


def _prep_core_inputs(node_pad, neib_bf, W1, W1b, M2, W_node, W_neib, st=ST_FULL):
    """Split padded full arrays into per-core input dicts."""
    ncn = st * NODES_ST
    ins = []
    for c in range(NCORES):
        ins.append(
            {
                "node": np.ascontiguousarray(node_pad[c * ncn : (c + 1) * ncn]),
                "neib": np.ascontiguousarray(
                    neib_bf[c * ncn * K : (c + 1) * ncn * K].reshape(st, CH, RP, D)
                ),
                "w1f": W1,
                "w1b": W1b,
                "m2": M2,
                "wnode": W_node,
                "wneib": W_neib,
                "sel4": _sel4_const(),
            }
        )
    return ins


def kernel(node_feats, neib_feats, node_ids, neib_ids, W_att1, W_att2, W_node, W_neib):
    from concourse.bass_utils import run_bass_kernel_spmd

    node_feats = np.asarray(node_feats, dtype=np.float32)
    neib_feats = np.asarray(neib_feats, dtype=np.float32)
    W1 = np.ascontiguousarray(np.asarray(W_att1, dtype=np.float32))
    W2 = np.asarray(W_att2, dtype=np.float32)
    W_node = np.ascontiguousarray(np.asarray(W_node, dtype=np.float32))
    W_neib = np.ascontiguousarray(np.asarray(W_neib, dtype=np.float32))
    M2 = (W2.astype(np.float64) @ W2.astype(np.float64).T).astype(np.float32)
    W1b = W1.astype(ml_dtypes.bfloat16)

    n = node_feats.shape[0]
    node_pad = np.zeros((NPAD, D), dtype=np.float32)
    node_pad[:n] = node_feats
    neib_bf = np.zeros((NPAD * K, D), dtype=ml_dtypes.bfloat16)
    neib_bf[: n * K] = neib_feats.astype(ml_dtypes.bfloat16)

    if "nc" not in _module_cache:
        _module_cache["nc"] = build_module(ST_FULL)
    nc = _module_cache["nc"]

    in_maps = _prep_core_inputs(node_pad, neib_bf, W1, W1b, M2, W_node, W_neib)
    res = run_bass_kernel_spmd(nc, in_maps, core_ids=list(range(NCORES)))
    outs = np.concatenate([res.results[c]["out"] for c in range(NCORES)], axis=0)
    return np.ascontiguousarray(outs[:n])



# revision 12
# speedup vs baseline: 1.1587x; 1.1587x over previous
"""AttentionAggregator Trainium2 kernel (8-core SPMD, data-parallel over nodes).

Reference computation (per node n, K=32 neighbors, D=128, H=32, O=128):
  att(x) = tanh(x @ W1) @ W2
  scores[n,k] = <att(neib[n,k]), att(node[n])>
  ws = softmax_k(scores);  agg[n] = sum_k ws[n,k] * neib[n,k]
  out = relu([node @ W_node, agg @ W_neib])

The device kernel is SYNC-BOUND, not compute/DMA-bound: per-iteration tile
choreography (TileContext semaphore edges) costs ~16 us regardless of work,
so the loop processes FAT iterations of 256 nodes (2 classic supertiles,
64 neighbor chunks of 128 rows) to halve the per-node sync overhead:
  * scores fold: <u W2, v W2> = u @ (W2 W2^T) @ v^T, M2 = W2 W2^T precomputed
    on host: scores[n,k] = u[n,k] . w[n], u = tanh(neib @ W1),
    w = tanh(node@W1) @ M2
  * neib is cast to bf16 on host and loaded twice from HBM: natural layout
    [rows, D] (moving operand of the aggregation matmuls) and XBAR-transposed
    [D, rows] (stationary of the att matmul). Loads are spread over the
    SP/Act HWDGE rings and the Pool SWDGE ring.
  * softmax runs max-free (tanh bounds |scores|) with deferred normalization:
    E = exp(scores); the aggregation matmul gets a 129th ones-column so
    Z = sum_k E arrives in the same PSUM tile; agg = agg_un * (1/Z).
  * aggregation: chunk t (nodes 4t..4t+3) uses a block-diagonal stationary
    Wsel[(j,k), c] = E[node 4t+j, k] * (c == 4*(t%8)+j) against the natural
    chunk; 8 chunks accumulate one 32-partition region of a [128, 132] PSUM
    tile (partition offset 32*((t//8)%4)), 4 regions per tile, 2 tiles per
    fat iteration; ONE reciprocal/normalize/transpose then covers 128 nodes.
  * w replication across K goes through a DRAM scratch (write [128,32] once,
    read back with a k-broadcast access pattern + sel4 PE broadcast).
"""

import sys

sys.path.insert(0, "/opt/trn_rl_repo")

import numpy as np
import ml_dtypes

N, K, D, H, O = 50000, 32, 128, 32, 128
NCORES = 8
ST_FULL = 50          # classic 128-node supertiles per core (padded)
NODES_ST = 128
CH = 32               # 128-row chunks per classic supertile
RP = 128              # rows per chunk
FAT = 2               # classic supertiles per loop iteration
NC_FULL = ST_FULL * NODES_ST          # 6400 nodes/core
NPAD = NC_FULL * NCORES               # 51200

_module_cache = {}


def _sel4_const():
    s = np.zeros((4, 128), dtype=ml_dtypes.bfloat16)
    for j in range(4):
        s[j, 32 * j : 32 * (j + 1)] = 1.0
    return s


def _patch_tile_drain():
    """This container's walrus rejects >1 sync-wait on one instruction; spread
    the TileContext tail-drain waits over extra sync nops."""
    from concourse import mybir
    from concourse import tile as tile_mod
    from concourse.tile import TileContext

    if getattr(TileContext, "_drain_patched", False):
        return
    MAXW = 1

    def _drain_and_barrier(self, tick_clock, wait_clock):
        drain_inst = self.nc.sync.drain()
        wait_clock.add_sem_waits(
            drain_inst.ins, tile_mod.ScopedClock({None: tick_clock.global_clock})
        )
        mi = drain_inst.ins
        ws = list(mi.sync_info.on_wait)
        if len(ws) > MAXW:
            mi.sync_info.on_wait = ws[:MAXW]
            rest = ws[MAXW:]
            for i in range(0, len(rest), MAXW):
                nop = self.nc.sync.nop(nofuse=True)
                nmi = nop.ins
                if nmi.sync_info is None:
                    nmi.sync_info = mybir.SyncInfo(
                        on_wait=rest[i : i + MAXW], on_update=[]
                    )
                else:
                    nmi.sync_info.on_wait = rest[i : i + MAXW]
        self.nc.all_engine_barrier()
        assert self.sems is not None
        popped = self.nc._tile_sem_poison_stack.pop()
        assert popped is self._sem_poison
        self.nc.clear_and_free_semaphores(list(self.sems.allocated().values()))
        self.nc.all_engine_barrier()

    TileContext._drain_and_barrier = _drain_and_barrier
    TileContext._drain_patched = True


def _split_multi_waits(nc, maxw=1):
    """Walrus in this container allows only one sync-wait per instruction:
    hoist extra waits onto same-engine NOPs inserted just before."""
    from concourse import mybir

    nsplit = 0
    for f in nc.m.functions:
        for b in f.blocks:
            changed = False
            out = []
            for inst in list(b.instructions):
                si = getattr(inst, "sync_info", None)
                ws = list(si.on_wait) if si is not None and si.on_wait else []
                if len(ws) > maxw:
                    keep = ws[-maxw:]
                    rest = ws[:-maxw]
                    for i in range(0, len(rest), maxw):
                        nop = mybir.InstNoOp(
                            name=f"I-wsplit{nc.next_id()}", ins=[], outs=[]
                        )
                        nop.engine = inst.engine
                        nop.sync_info = mybir.SyncInfo(
                            on_wait=rest[i : i + maxw], on_update=[]
                        )
                        out.append(nop)
                    si.on_wait = keep
                    changed = True
                    nsplit += 1
                out.append(inst)
            if changed:
                b.instructions = out
    return nsplit


def build_module(st=ST_FULL, ablate=(), repeat=1, bufs_bigs=3, bufs_mids=3, bufs_uw=4,
                 aggmode="merged"):
    import concourse.bass as bass
    from concourse import mybir
    from concourse.tile import TileContext
    from concourse.masks import make_identity

    ablate = set(ablate)
    _patch_tile_drain()

    f32 = mybir.dt.float32
    bf16 = mybir.dt.bfloat16
    AF = mybir.ActivationFunctionType
    ALU = mybir.AluOpType
    assert st % FAT == 0
    nit = st // FAT          # fat iterations
    CH2 = CH * FAT           # 64 chunks per fat iteration
    NOD2 = NODES_ST * FAT    # 256 nodes per fat iteration
    ncn = st * NODES_ST

    nc = bass.Bass()
    node = nc.declare_dram_parameter("node", [ncn, D], f32, isOutput=False)
    neib = nc.declare_dram_parameter("neib", [st, CH, RP, D], bf16, isOutput=False)
    w1f = nc.declare_dram_parameter("w1f", [D, H], f32, isOutput=False)
    w1b = nc.declare_dram_parameter("w1b", [D, H], bf16, isOutput=False)
    m2 = nc.declare_dram_parameter("m2", [H, H], f32, isOutput=False)
    wnode = nc.declare_dram_parameter("wnode", [D, O], f32, isOutput=False)
    wneib = nc.declare_dram_parameter("wneib", [D, O], f32, isOutput=False)
    sel4p = nc.declare_dram_parameter("sel4", [4, 128], bf16, isOutput=False)
    out = nc.declare_dram_parameter("out", [ncn, 2 * O], f32, isOutput=True)
    # w scratch in DRAM: [classic supertile, node-in-supertile, h]
    wscr = nc.dram_tensor("wscr", [st, NODES_ST, H], bf16)

    with TileContext(nc) as tc:
        with (
            tc.tile_pool(name="singles", bufs=1) as singles,
            tc.tile_pool(name="nodep", bufs=3) as nodep,
            tc.tile_pool(name="bigs", bufs=bufs_bigs) as bigs,
            tc.tile_pool(name="mids", bufs=bufs_mids) as mids,
            tc.tile_pool(name="outs", bufs=3) as outs,
            tc.tile_pool(name="ps_uw", bufs=bufs_uw, space="PSUM") as ps_uw,
            tc.tile_pool(name="ps_agg", bufs=2, space="PSUM") as ps_agg,
            tc.tile_pool(name="ps_small", bufs=2, space="PSUM") as ps_small,
        ):
            # ---- one-time constants ----
            ident128 = singles.tile([128, 128], f32)
            make_identity(nc, ident128)
            w1f_sb = singles.tile([D, H], f32)
            nc.gpsimd.dma_start(out=w1f_sb, in_=w1f[:, :])
            w1b_sb = singles.tile([D, H], bf16)
            nc.gpsimd.dma_start(out=w1b_sb, in_=w1b[:, :])
            m2_sb = singles.tile([H, H], f32)
            nc.gpsimd.dma_start(out=m2_sb, in_=m2[:, :])
            wnode_sb = singles.tile([D, O], f32)
            nc.gpsimd.dma_start(out=wnode_sb, in_=wnode[:, :])
            wneib_sb = singles.tile([D, O], f32)
            nc.gpsimd.dma_start(out=wneib_sb, in_=wneib[:, :])
            # maskW[p, tm, c] = 1 if c == 4*tm + p//32 else 0   (bf16)
            maskw = singles.tile([128, 8, 32], bf16)
            nc.vector.memset(maskw, 0.0)
            for tm in range(8):
                for j in range(4):
                    nc.vector.memset(
                        maskw[32 * j : 32 * j + 32, tm : tm + 1, 4 * tm + j : 4 * tm + j + 1],
                        1.0,
                    )
            # sel4[j, p] = 1 if p//32 == j (partition-broadcast selector)
            sel4 = singles.tile([4, 128], bf16)
            nc.gpsimd.dma_start(out=sel4, in_=sel4p[:, :])

            out_tiles = {}

            def node_path(i):
                """nodes 256i..256i+255: out[:,0:128] and w=tanh(node@W1)@M2."""
                node_sb = nodep.tile([128, FAT, D], f32, tag="node_sb")
                nc.scalar.dma_start(
                    out=node_sb,
                    in_=node[i * NOD2 : (i + 1) * NOD2, :].rearrange(
                        "(o p) d -> p o d", o=FAT
                    ),
                )
                nodeT_ps = ps_small.tile([128, FAT, 128], f32, tag="small")
                for o in range(FAT):
                    nc.tensor.transpose(
                        nodeT_ps[:, o, :], node_sb[:, o, :], ident128
                    )
                nodeT_sb = nodep.tile([128, FAT, 128], f32, tag="nodeT_sb")
                nc.scalar.copy(nodeT_sb, nodeT_ps)
                # out1 = relu(node @ W_node), per 128-node half
                out_sb = outs.tile([128, FAT, 2 * O], f32, tag="out_sb")
                out_tiles[i] = out_sb
                for o in range(FAT):
                    out1_ps = ps_small.tile([128, O], f32, tag="small")
                    nc.tensor.matmul(out1_ps, lhsT=nodeT_sb[:, o, :], rhs=wnode_sb)
                    nc.scalar.activation(out_sb[:, o, 0:O], out1_ps, AF.Relu)
                # vT = tanh(W1^T @ nodeT) : [H, 256] in one matmul
                vT_ps = ps_small.tile([H, FAT * 128], f32, tag="small")
                nc.tensor.matmul(
                    vT_ps,
                    lhsT=w1f_sb,
                    rhs=nodeT_sb[:, :, :].rearrange("p o n -> p (o n)"),
                )
                vT_sb = nodep.tile([H, FAT, 128], f32, tag="vT_sb")
                nc.scalar.activation(
                    vT_sb, vT_ps[:, :].rearrange("p (o n) -> p o n", o=FAT), AF.Tanh
                )
                # w = v @ M2 : [128, H] per half -> wscr
                w_sb = nodep.tile([128, FAT, H], bf16, tag="w_sb")
                for o in range(FAT):
                    w_ps = ps_small.tile([128, H], f32, tag="small")
                    nc.tensor.matmul(w_ps, lhsT=vT_sb[:, o, :], rhs=m2_sb)
                    nc.scalar.copy(w_sb[:, o, :], w_ps)
                nc.sync.dma_start(
                    out=wscr[FAT * i : FAT * (i + 1), :, :].rearrange(
                        "o p h -> p o h"
                    ),
                    in_=w_sb,
                )

            def main_path(i):
                """neighbor attention + aggregation for fat iteration i."""
                # DMA ring balancing across the two HWDGE rings (SP/Act):
                # each carries one big load per iteration. The Pool SWDGE
                # ring is NOT safe for in-loop loads here: once it lags the
                # consumers race ahead of the data (observed whole-iteration
                # corruption for the iterations whose load fell behind).
                nat_eng = nc.scalar if i % 2 == 0 else nc.sync
                xb_eng = nc.sync if i % 2 == 0 else nc.scalar
                # natural bf16 load: nb[p, t, 0:128] = neib row; col 128 = 1.0
                nb = bigs.tile([128, CH2, 132], bf16, tag="nb")
                if "nb" not in ablate:
                    nat_eng.dma_start(
                        out=nb[:, :, 0:D],
                        in_=neib[FAT * i : FAT * (i + 1), :, :, :].rearrange(
                            "o t p d -> p (o t) d"
                        ),
                    )
                nc.vector.memset(nb[:, :, D : D + 1], 1.0)
                # XBAR-transposed load: nbT[d, 128*t + p]
                nbT = bigs.tile([128, CH2 * RP], bf16, tag="nbT")
                if "noxbar" in ablate:
                    nc.vector.memset(nbT[:, 0:4], 0.0)
                else:
                    xb_eng.dma_start(
                        out=nbT,
                        in_=neib[FAT * i : FAT * (i + 1), :, :, :].rearrange(
                            "o t p d -> (o t p) d"
                        ),
                        transpose=True,
                    )
                # u = tanh(neib @ W1) in natural layout, 16 chunks per PSUM tile
                u_sb = mids.tile([128, CH2, H], bf16, tag="u")
                if "umm" in ablate:
                    nc.vector.memset(u_sb[:, 0:1, :], 0.0)
                for q in range(CH2 // 16) if "umm" not in ablate else ():
                    u_ps = ps_uw.tile([128, 16 * H], f32, tag="uw")
                    for tt in range(16):
                        t = 16 * q + tt
                        nc.tensor.matmul(
                            u_ps[:, tt * H : (tt + 1) * H],
                            lhsT=nbT[:, t * RP : (t + 1) * RP],
                            rhs=w1b_sb,
                        )
                    nc.scalar.activation(
                        u_sb[:, 16 * q : 16 * (q + 1), :],
                        u_ps[:, :].rearrange("p (t h) -> p t h", h=H),
                        AF.Tanh,
                    )
                # w replicated over k: wrep[32j+k, t, h] = w[4t+j, h].
                wrep = mids.tile([128, CH2, H], bf16, tag="wrep")
                if "wrep" in ablate:
                    nc.vector.memset(wrep[:, 0:1, :], 0.0)
                else:
                    w4 = mids.tile([4, CH2, H], bf16, tag="w4")
                    for o in range(FAT):
                        base = wscr[FAT * i + o : FAT * i + o + 1, 0:1, 0:1]
                        in_ap = bass.AP(
                            tensor=base.tensor,
                            offset=base.offset,
                            ap=[[H, 4], [4 * H, CH], [1, H]],
                        )
                        nc.sync.dma_start(out=w4[:, CH * o : CH * (o + 1), :], in_=in_ap)
                    w4f = w4[:, :, :].rearrange("j t h -> j (t h)")
                    for hh in range(CH2 * H // 512):
                        wrep_ps = ps_uw.tile([128, 512], f32, tag="uw")
                        nc.tensor.matmul(
                            wrep_ps, lhsT=sel4, rhs=w4f[:, 512 * hh : 512 * (hh + 1)]
                        )
                        nc.scalar.copy(
                            wrep[:, 16 * hh : 16 * (hh + 1), :],
                            wrep_ps[:, :].rearrange("p (t h) -> p t h", h=H),
                        )
                # scores[p, t] = sum_h u[p,t,h] * wrep[p,t,h]
                tmp = mids.tile([128, CH2, H], bf16, tag="tmp")
                scores = mids.tile([128, CH2], f32, tag="scores")
                e_sb = mids.tile([128, CH2], bf16, tag="e")
                wsel = mids.tile([128, CH2, 32], bf16, tag="wsel")
                if "score" in ablate:
                    nc.vector.memset(scores, 0.0)
                    nc.vector.memset(e_sb, 1.0)
                    nc.vector.memset(wsel[:, :, 0:1], 1.0)
                else:
                    nc.vector.tensor_mul(tmp, u_sb, wrep)
                    nc.vector.tensor_reduce(
                        scores, tmp, axis=mybir.AxisListType.X, op=ALU.add
                    )
                    nc.scalar.activation(e_sb, scores, AF.Exp)
                    # wsel[p, (g,tm), c] = E[p, (g,tm)] * maskW[p, tm, c]
                    e_ap = e_sb[:, :]
                    e_b = bass.AP(
                        tensor=e_ap.tensor,
                        offset=e_ap.offset,
                        ap=[
                            e_ap.ap[0],
                            [8 * e_ap.ap[1][0], CH2 // 8],
                            [e_ap.ap[1][0], 8],
                            [0, 32],
                        ],
                    )
                    m_ap = maskw[:, :, :]
                    m_b = bass.AP(
                        tensor=m_ap.tensor,
                        offset=m_ap.offset,
                        ap=[m_ap.ap[0], [0, CH2 // 8], m_ap.ap[1], m_ap.ap[2]],
                    )
                    wsel_v = wsel[:, :, :].rearrange(
                        "p (g tm) c -> p g tm c", tm=8
                    )
                    nc.vector.tensor_tensor(wsel_v, e_b, m_b, op=ALU.mult)
                # aggregation: chunk t -> psum tile t//32, partition region
                # 32*((t//8)%4); 8 chunks accumulate each region. One
                # normalize + transpose then covers 128 nodes.
                out_sb = out_tiles.pop(i, None)
                if out_sb is None:  # node path ablated
                    out_sb = outs.tile([128, FAT, 2 * O], f32, tag="out_sb")
                for half in range(FAT):
                    if aggmode == "split":
                        agg_ps = None
                        agg_sb_s = mids.tile([128, D], f32, tag="agg_sb")
                        z_sb = mids.tile([128, 1], f32, tag="z")
                        for a in range(4):
                            agg_ps_a = ps_agg.tile([32, 132], f32, tag="agg")
                            for b in range(8) if "agg" not in ablate else ():
                                t = 32 * half + 8 * a + b
                                nc.tensor.matmul(
                                    agg_ps_a[:, 0 : D + 1],
                                    lhsT=wsel[:, t : t + 1, :],
                                    rhs=nb[:, t : t + 1, 0 : D + 1],
                                    start=(b == 0),
                                    stop=(b == 7),
                                )
                            if "agg" in ablate:
                                nc.vector.memset(agg_ps_a[:, 0:132], 0.0)
                            nc.vector.tensor_copy(
                                agg_sb_s[32 * a : 32 * a + 32, :], agg_ps_a[:, 0:D]
                            )
                            nc.vector.tensor_copy(
                                z_sb[32 * a : 32 * a + 32, :], agg_ps_a[:, D : D + 1]
                            )
                    else:
                        agg_ps = ps_agg.tile([128, 132], f32, tag="agg")
                        if "agg" in ablate:
                            nc.vector.memset(agg_ps[:, 0:132], 0.0)
                        for tq in range(32) if "agg" not in ablate else ():
                            t = 32 * half + tq
                            nc.tensor.matmul(
                                agg_ps[32 * (tq // 8) : 32 * (tq // 8) + 32, 0 : D + 1],
                                lhsT=wsel[:, t : t + 1, :],
                                rhs=nb[:, t : t + 1, 0 : D + 1],
                                start=(tq % 8 == 0),
                                stop=(tq % 8 == 7),
                                tile_position=(0, 32 * (tq // 8)),
                            )
                    if "norm" in ablate:
                        aggT_sb = mids.tile([128, 128], f32, tag="aggT_sb")
                        nc.vector.memset(aggT_sb[:, 0:4], 0.0)
                    else:
                        rz = mids.tile([128, 1], f32, tag="rz")
                        agg_sb = mids.tile([128, D], f32, tag="agg_sb2")
                        if aggmode == "split":
                            nc.vector.reciprocal(rz, z_sb)
                            nc.vector.tensor_scalar(
                                agg_sb, agg_sb_s, rz, None, op0=ALU.mult
                            )
                        else:
                            nc.vector.reciprocal(rz, agg_ps[:, D : D + 1])
                            nc.vector.tensor_scalar(
                                agg_sb, agg_ps[:, 0:D], rz, None, op0=ALU.mult
                            )
                        aggT_ps = ps_small.tile([128, 128], f32, tag="small")
                        nc.tensor.transpose(aggT_ps, agg_sb, ident128)
                        aggT_sb = mids.tile([128, 128], f32, tag="aggT_sb")
                        nc.scalar.copy(aggT_sb, aggT_ps)
                    out2_ps = ps_small.tile([128, O], f32, tag="small")
                    nc.tensor.matmul(out2_ps, lhsT=aggT_sb, rhs=wneib_sb)
                    nc.scalar.activation(out_sb[:, half, O : 2 * O], out2_ps, AF.Relu)
                if "outw" not in ablate:
                    nc.scalar.dma_start(
                        out=out[i * NOD2 : (i + 1) * NOD2, :].rearrange(
                            "(o p) c -> p o c", o=FAT
                        ),
                        in_=out_sb,
                    )

            for _rep in range(repeat):
                for i in range(nit + 1):
                    if i < nit and "nodep" not in ablate:
                        node_path(i)
                    if i >= 1:
                        main_path(i - 1)

    _split_multi_waits(nc)
    return nc


def _prep_core_inputs(node_pad, neib_bf, W1, W1b, M2, W_node, W_neib, st=ST_FULL):
    """Split padded full arrays into per-core input dicts."""
    ncn = st * NODES_ST
    ins = []
    for c in range(NCORES):
        ins.append(
            {
                "node": np.ascontiguousarray(node_pad[c * ncn : (c + 1) * ncn]),
                "neib": np.ascontiguousarray(
                    neib_bf[c * ncn * K : (c + 1) * ncn * K].reshape(st, CH, RP, D)
                ),
                "w1f": W1,
                "w1b": W1b,
                "m2": M2,
                "wnode": W_node,
                "wneib": W_neib,
                "sel4": _sel4_const(),
            }
        )
    return ins


def kernel(node_feats, neib_feats, node_ids, neib_ids, W_att1, W_att2, W_node, W_neib):
    from concourse.bass_utils import run_bass_kernel_spmd

    node_feats = np.asarray(node_feats, dtype=np.float32)
    neib_feats = np.asarray(neib_feats, dtype=np.float32)
    W1 = np.ascontiguousarray(np.asarray(W_att1, dtype=np.float32))
    W2 = np.asarray(W_att2, dtype=np.float32)
    W_node = np.ascontiguousarray(np.asarray(W_node, dtype=np.float32))
    W_neib = np.ascontiguousarray(np.asarray(W_neib, dtype=np.float32))
    M2 = (W2.astype(np.float64) @ W2.astype(np.float64).T).astype(np.float32)
    W1b = W1.astype(ml_dtypes.bfloat16)

    n = node_feats.shape[0]
    node_pad = np.zeros((NPAD, D), dtype=np.float32)
    node_pad[:n] = node_feats
    neib_bf = np.zeros((NPAD * K, D), dtype=ml_dtypes.bfloat16)
    neib_bf[: n * K] = neib_feats.astype(ml_dtypes.bfloat16)

    if "nc" not in _module_cache:
        _module_cache["nc"] = build_module(ST_FULL)
    nc = _module_cache["nc"]

    in_maps = _prep_core_inputs(node_pad, neib_bf, W1, W1b, M2, W_node, W_neib)
    res = run_bass_kernel_spmd(nc, in_maps, core_ids=list(range(NCORES)))
    outs = np.concatenate([res.results[c]["out"] for c in range(NCORES)], axis=0)
    return np.ascontiguousarray(outs[:n])


# revision 15
# speedup vs baseline: 1.2833x; 1.1075x over previous
"""AttentionAggregator Trainium2 kernel (8-core SPMD, data-parallel over nodes).

Reference computation (per node n, K=32 neighbors, D=128, H=32, O=128):
  att(x) = tanh(x @ W1) @ W2
  scores[n,k] = <att(neib[n,k]), att(node[n])>
  ws = softmax_k(scores);  agg[n] = sum_k ws[n,k] * neib[n,k]
  out = relu([node @ W_node, agg @ W_neib])

Device strategy (per core: 6272 nodes = 49 supertiles of 128 nodes; each
supertile = 4096 neighbor rows = 32 chunks of 128 rows):
  * scores fold: <u W2, v W2> = u @ (W2 W2^T) @ v^T, M2 = W2 W2^T precomputed
    on host, so the per-neighbor att2 matmul disappears:
    scores[n,k] = u[n,k] . w[n],  u = tanh(neib @ W1),  w = tanh(node@W1) @ M2
  * neib is cast to bf16 on host and loaded twice from HBM: natural layout
    [rows, D] (moving operand of the aggregation matmuls) and XBAR-transposed
    [D, rows] (stationary of the att matmul, which then emits u in natural
    [rows, H] layout so softmax/score work runs at full 128-partition width).
  * softmax runs max-free (tanh bounds |scores|) with deferred normalization:
    E = exp(scores); the aggregation matmul gets a 129th ones-column so
    Z = sum_k E arrives in the same PSUM tile; agg = agg_un * (1/Z).
  * aggregation: per chunk t (nodes 4t..4t+3) a block-diagonal stationary
    Wsel[(j,k), c] = E[node 4t+j, k] * (c == 4*(t%8)+j) against the natural
    chunk; 8 chunks accumulate a [32 nodes, 129] PSUM tile; 4 groups per
    supertile.
  * w replication across K goes through a DRAM scratch (write [128,32] once,
    read back with a k-broadcast access pattern), since cross-partition
    replication is not expressible on the compute engines.
"""

import sys

sys.path.insert(0, "/opt/trn_rl_repo")

import numpy as np
import ml_dtypes

N, K, D, H, O = 50000, 32, 128, 32, 128
NCORES = 8
ST_FULL = 49          # supertiles per core
NODES_ST = 128        # nodes per supertile
CH = 32               # 128-row chunks per supertile
RP = 128              # rows per chunk
NC_FULL = ST_FULL * NODES_ST          # 6272 nodes/core
NPAD = NC_FULL * NCORES               # 50176

_module_cache = {}


def _sel4_const():
    s = np.zeros((4, 128), dtype=ml_dtypes.bfloat16)
    for j in range(4):
        s[j, 32 * j : 32 * (j + 1)] = 1.0
    return s


def _patch_tile_drain():
    """This container's walrus rejects >1 sync-wait on one instruction; spread
    the TileContext tail-drain waits over extra sync nops."""
    from concourse import mybir
    from concourse import tile as tile_mod
    from concourse.tile import TileContext

    if getattr(TileContext, "_drain_patched", False):
        return
    MAXW = 1

    def _drain_and_barrier(self, tick_clock, wait_clock):
        drain_inst = self.nc.sync.drain()
        wait_clock.add_sem_waits(
            drain_inst.ins, tile_mod.ScopedClock({None: tick_clock.global_clock})
        )
        mi = drain_inst.ins
        ws = list(mi.sync_info.on_wait)
        if len(ws) > MAXW:
            mi.sync_info.on_wait = ws[:MAXW]
            rest = ws[MAXW:]
            for i in range(0, len(rest), MAXW):
                nop = self.nc.sync.nop(nofuse=True)
                nmi = nop.ins
                if nmi.sync_info is None:
                    nmi.sync_info = mybir.SyncInfo(
                        on_wait=rest[i : i + MAXW], on_update=[]
                    )
                else:
                    nmi.sync_info.on_wait = rest[i : i + MAXW]
        self.nc.all_engine_barrier()
        assert self.sems is not None
        popped = self.nc._tile_sem_poison_stack.pop()
        assert popped is self._sem_poison
        self.nc.clear_and_free_semaphores(list(self.sems.allocated().values()))
        self.nc.all_engine_barrier()

    TileContext._drain_and_barrier = _drain_and_barrier
    TileContext._drain_patched = True


def _split_multi_waits(nc, maxw=1):
    """Walrus in this container allows only one sync-wait per instruction:
    hoist extra waits onto same-engine NOPs inserted just before."""
    from concourse import mybir

    nsplit = 0
    for f in nc.m.functions:
        for b in f.blocks:
            changed = False
            out = []
            for inst in list(b.instructions):
                si = getattr(inst, "sync_info", None)
                ws = list(si.on_wait) if si is not None and si.on_wait else []
                if len(ws) > maxw:
                    keep = ws[-maxw:]
                    rest = ws[:-maxw]
                    for i in range(0, len(rest), maxw):
                        nop = mybir.InstNoOp(
                            name=f"I-wsplit{nc.next_id()}", ins=[], outs=[]
                        )
                        nop.engine = inst.engine
                        nop.sync_info = mybir.SyncInfo(
                            on_wait=rest[i : i + maxw], on_update=[]
                        )
                        out.append(nop)
                    si.on_wait = keep
                    changed = True
                    nsplit += 1
                out.append(inst)
            if changed:
                b.instructions = out
    return nsplit


def build_module(st=ST_FULL, ablate=(), repeat=1, bufs_bigs=3, bufs_mids=3, bufs_uw=4):
    import concourse.bass as bass
    from concourse import mybir
    from concourse.tile import TileContext
    from concourse.masks import make_identity

    ablate = set(ablate)
    _patch_tile_drain()

    f32 = mybir.dt.float32
    bf16 = mybir.dt.bfloat16
    AF = mybir.ActivationFunctionType
    ALU = mybir.AluOpType
    ncn = st * NODES_ST  # nodes this build handles per core

    nc = bass.Bass()
    node = nc.declare_dram_parameter("node", [ncn, D], f32, isOutput=False)
    neib = nc.declare_dram_parameter("neib", [st, CH, RP, D], bf16, isOutput=False)
    w1f = nc.declare_dram_parameter("w1f", [D, H], f32, isOutput=False)
    w1b = nc.declare_dram_parameter("w1b", [D, H], bf16, isOutput=False)
    m2 = nc.declare_dram_parameter("m2", [H, H], f32, isOutput=False)
    wnode = nc.declare_dram_parameter("wnode", [D, O], f32, isOutput=False)
    wneib = nc.declare_dram_parameter("wneib", [D, O], f32, isOutput=False)
    sel4p = nc.declare_dram_parameter("sel4", [4, 128], bf16, isOutput=False)
    out = nc.declare_dram_parameter("out", [ncn, 2 * O], f32, isOutput=True)
    # w scratch in DRAM: [supertile, node-in-supertile, h]
    wscr = nc.dram_tensor("wscr", [st, NODES_ST, H], bf16)

    with TileContext(nc) as tc:
        with (
            tc.tile_pool(name="singles", bufs=1) as singles,
            tc.tile_pool(name="nodep", bufs=3) as nodep,
            tc.tile_pool(name="bigs", bufs=bufs_bigs) as bigs,
            tc.tile_pool(name="mids", bufs=bufs_mids) as mids,
            tc.tile_pool(name="outs", bufs=3) as outs,
            tc.tile_pool(name="ps_uw", bufs=bufs_uw, space="PSUM") as ps_uw,
            tc.tile_pool(name="ps_agg", bufs=2, space="PSUM") as ps_agg,
            tc.tile_pool(name="ps_small", bufs=2, space="PSUM") as ps_small,
        ):
            # ---- one-time constants ----
            ident128 = singles.tile([128, 128], f32)
            make_identity(nc, ident128)
            ident32 = singles.tile([32, 32], f32)
            make_identity(nc, ident32)
            w1f_sb = singles.tile([D, H], f32)
            nc.gpsimd.dma_start(out=w1f_sb, in_=w1f[:, :])
            w1b_sb = singles.tile([D, H], bf16)
            nc.gpsimd.dma_start(out=w1b_sb, in_=w1b[:, :])
            m2_sb = singles.tile([H, H], f32)
            nc.gpsimd.dma_start(out=m2_sb, in_=m2[:, :])
            wnode_sb = singles.tile([D, O], f32)
            nc.gpsimd.dma_start(out=wnode_sb, in_=wnode[:, :])
            wneib_sb = singles.tile([D, O], f32)
            nc.gpsimd.dma_start(out=wneib_sb, in_=wneib[:, :])
            # maskW[p, tm, c] = 1 if c == 4*tm + p//32 else 0   (bf16)
            maskw = singles.tile([128, 8, 32], bf16)
            nc.vector.memset(maskw, 0.0)
            for tm in range(8):
                for j in range(4):
                    nc.vector.memset(
                        maskw[32 * j : 32 * j + 32, tm : tm + 1, 4 * tm + j : 4 * tm + j + 1],
                        1.0,
                    )
            ones1 = singles.tile([128, 1], bf16)
            nc.vector.memset(ones1, 1.0)
            # sel4[j, p] = 1 if p//32 == j (partition-broadcast selector)
            sel4 = singles.tile([4, 128], bf16)
            nc.gpsimd.dma_start(out=sel4, in_=sel4p[:, :])

            out_tiles = {}

            def node_path(s):
                """node tile s: out[:,0:128], and w[n]=tanh(node@W1)@M2 -> wscr[s]."""
                node_sb = nodep.tile([128, D], f32, tag="node_sb")
                nc.scalar.dma_start(out=node_sb, in_=node[s * 128 : (s + 1) * 128, :])
                nodeT_ps = ps_small.tile([128, 128], f32, tag="small")
                nc.tensor.transpose(nodeT_ps, node_sb, ident128)
                nodeT_sb = nodep.tile([128, 128], f32, tag="nodeT_sb")
                nc.scalar.copy(nodeT_sb, nodeT_ps)
                # out1 = relu(node @ W_node)
                out1_ps = ps_small.tile([128, O], f32, tag="small")
                nc.tensor.matmul(out1_ps, lhsT=nodeT_sb, rhs=wnode_sb)
                out_sb = outs.tile([128, 2 * O], f32, tag="out_sb")
                out_tiles[s] = out_sb
                nc.scalar.activation(out_sb[:, 0:O], out1_ps, AF.Relu)
                # vT = tanh(W1^T @ nodeT) : [H, 128]
                vT_ps = ps_small.tile([H, 128], f32, tag="small")
                nc.tensor.matmul(vT_ps, lhsT=w1f_sb, rhs=nodeT_sb)
                vT_sb = nodep.tile([H, 128], f32, tag="vT_sb")
                nc.scalar.activation(vT_sb, vT_ps, AF.Tanh)
                # w = v @ M2 : [128, H]
                w_ps = ps_small.tile([128, H], f32, tag="small")
                nc.tensor.matmul(w_ps, lhsT=vT_sb, rhs=m2_sb)
                w_sb = nodep.tile([128, H], bf16, tag="w_sb")
                nc.scalar.copy(w_sb, w_ps)
                nc.sync.dma_start(out=wscr[s : s + 1, :, :], in_=w_sb)

            def main_path(s):
                """neighbor attention + aggregation for supertile s."""
                # DMA ring balancing: the XBAR-transposed load can only run on
                # the two HWDGE rings (SP/Act), so split it between them; the
                # natural-layout load goes mostly to the Pool SWDGE ring (with
                # a slice to SP) so no single ring carries more than ~38 MB.
                # NOTE: the ring assignment below is load-bearing for
                # CORRECTNESS, not just speed. The wscr DRAM scratch
                # write->read (node_path -> w4 load) is not tracked by the
                # tile framework; it stays safe only because both sit on the
                # SP ring in this exact order with a full iteration of slack.
                # Every attempted rebalance (natural load on Pool SWDGE, or
                # alternating the two HWDGE rings) produced nondeterministic
                # corruption: rings complete out of order on parallel DMA
                # engines, and consumers race ahead of late data.
                nat_eng = nc.scalar
                xb_eng = nc.sync
                # natural bf16 load: nb[p, t, 0:128] = neib row (s,t,p); col 128 = 1.0
                nb = bigs.tile([128, CH, 132], bf16, tag="nb")
                if "nb" not in ablate:
                    nat_eng.dma_start(
                        out=nb[:, :, 0:D],
                        in_=neib[s : s + 1, :, :, :].rearrange("o t p d -> p (o t) d"),
                    )
                nc.vector.memset(nb[:, :, D : D + 1], 1.0)
                # XBAR-transposed load: nbT[d, 128*t + p]
                nbT = bigs.tile([128, CH * RP], bf16, tag="nbT")
                if "noxbar" in ablate:
                    nc.vector.memset(nbT[:, 0:4], 0.0)
                elif "xbar" in ablate:
                    xb_eng.dma_start(
                        out=nbT,
                        in_=neib[s : s + 1, :, :, :].rearrange("o t p d -> p (o t) d"),
                    )
                else:
                    xb_eng.dma_start(
                        out=nbT,
                        in_=neib[s : s + 1, :, :, :].rearrange("o t p d -> (o t p) d"),
                        transpose=True,
                    )
                # u = tanh(neib @ W1) in natural layout, chunk by chunk.
                u_sb = mids.tile([128, CH, H], bf16, tag="u")
                if "umm" in ablate:
                    nc.vector.memset(u_sb[:, 0:1, :], 0.0)
                for half in range(2) if "umm" not in ablate else ():
                    u_ps = ps_uw.tile([128, 16 * H], f32, tag="uw")
                    for tt in range(16):
                        t = 16 * half + tt
                        nc.tensor.matmul(
                            u_ps[:, tt * H : (tt + 1) * H],
                            lhsT=nbT[:, t * RP : (t + 1) * RP],
                            rhs=w1b_sb,
                        )
                    nc.scalar.activation(
                        u_sb[:, 16 * half : 16 * (half + 1), :],
                        u_ps[:, :].rearrange("p (t h) -> p t h", h=H),
                        AF.Tanh,
                    )
                # w replicated over k: wrep[32j+k, t, h] = w[4t+j, h].
                # Tiny DRAM load w4[j, t, h] = w[4t+j, h], then PE broadcast
                # via sel4 (out[p] = w4[p//32]).
                wrep = mids.tile([128, CH, H], bf16, tag="wrep")
                if "wrep" in ablate:
                    nc.vector.memset(wrep[:, 0:1, :], 0.0)
                else:
                    w4 = mids.tile([4, CH, H], bf16, tag="w4")
                    base = wscr[s : s + 1, 0:1, 0:1]
                    in_ap = bass.AP(
                        tensor=base.tensor,
                        offset=base.offset,
                        ap=[[H, 4], [4 * H, CH], [1, H]],
                    )
                    nc.sync.dma_start(out=w4, in_=in_ap)
                    w4f = w4[:, :, :].rearrange("j t h -> j (t h)")
                    for hh in range(2):
                        wrep_ps = ps_uw.tile([128, 512], f32, tag="uw")
                        nc.tensor.matmul(
                            wrep_ps, lhsT=sel4, rhs=w4f[:, 512 * hh : 512 * (hh + 1)]
                        )
                        nc.scalar.copy(
                            wrep[:, 16 * hh : 16 * (hh + 1), :],
                            wrep_ps[:, :].rearrange("p (t h) -> p t h", h=H),
                        )
                # scores[p, t] = sum_h u[p,t,h] * wrep[p,t,h]
                tmp = mids.tile([128, CH, H], bf16, tag="tmp")
                scores = mids.tile([128, CH], f32, tag="scores")
                e_sb = mids.tile([128, CH], bf16, tag="e")
                wsel = mids.tile([128, CH, 32], bf16, tag="wsel")
                if "score" in ablate:
                    nc.vector.memset(scores, 0.0)
                    nc.vector.memset(e_sb, 1.0)
                    nc.vector.memset(wsel[:, :, 0:1], 1.0)
                else:
                    nc.vector.tensor_mul(tmp, u_sb, wrep)
                    nc.vector.tensor_reduce(
                        scores, tmp, axis=mybir.AxisListType.X, op=ALU.add
                    )
                    nc.scalar.activation(e_sb, scores, AF.Exp)
                    # wsel[p, (g,tm), c] = E[p, (g,tm)] * maskW[p, tm, c]
                    e_ap = e_sb[:, :]
                    e_b = bass.AP(
                        tensor=e_ap.tensor,
                        offset=e_ap.offset,
                        ap=[e_ap.ap[0], [8 * e_ap.ap[1][0], 4], [e_ap.ap[1][0], 8], [0, 32]],
                    )
                    m_ap = maskw[:, :, :]
                    m_b = bass.AP(
                        tensor=m_ap.tensor,
                        offset=m_ap.offset,
                        ap=[m_ap.ap[0], [0, 4], m_ap.ap[1], m_ap.ap[2]],
                    )
                    wsel_v = wsel[:, :, :].rearrange("p (g tm) c -> p g tm c", g=4)
                    nc.vector.tensor_tensor(wsel_v, e_b, m_b, op=ALU.mult)
                # aggregation: 4 groups of 8 chunks -> [32 nodes, 129] PSUM,
                # then normalize by the ones-column sum and transpose into aggT.
                aggT_ps = ps_small.tile([128, 128], f32, tag="small")
                for g in range(4):
                    agg_ps = ps_agg.tile([32, 132], f32, tag="agg")
                    if "agg" in ablate:
                        nc.vector.memset(agg_ps[:, 0:132], 0.0)
                    for tm in range(8) if "agg" not in ablate else ():
                        t = 8 * g + tm
                        nc.tensor.matmul(
                            agg_ps[:, 0 : D + 1],
                            lhsT=wsel[:, t : t + 1, :],
                            rhs=nb[:, t : t + 1, 0 : D + 1],
                            start=(tm == 0),
                            stop=(tm == 7),
                        )
                    if "norm" in ablate:
                        continue
                    rz = mids.tile([32, 1], f32, tag="rz")
                    nc.vector.reciprocal(rz, agg_ps[:, D : D + 1])
                    agg_sb = mids.tile([32, D], f32, tag="agg_sb")
                    nc.vector.tensor_scalar(
                        agg_sb, agg_ps[:, 0:D], rz, None, op0=ALU.mult
                    )
                    nc.tensor.transpose(
                        aggT_ps[:, 32 * g : 32 * (g + 1)], agg_sb, ident32
                    )
                aggT_sb = mids.tile([128, 128], f32, tag="aggT_sb")
                if "norm" in ablate:
                    nc.vector.memset(aggT_sb[:, 0:4], 0.0)
                else:
                    nc.scalar.copy(aggT_sb, aggT_ps)
                out2_ps = ps_small.tile([128, O], f32, tag="small")
                nc.tensor.matmul(out2_ps, lhsT=aggT_sb, rhs=wneib_sb)
                out_sb = out_tiles.pop(s, None)
                if out_sb is None:  # node path ablated
                    out_sb = outs.tile([128, 2 * O], f32, tag="out_sb")
                nc.scalar.activation(out_sb[:, O : 2 * O], out2_ps, AF.Relu)
                if "outw" not in ablate:
                    nc.scalar.dma_start(
                        out=out[s * 128 : (s + 1) * 128, :], in_=out_sb
                    )

            for _rep in range(repeat):
                for i in range(st + 1):
                    if i < st and "nodep" not in ablate:
                        node_path(i)
                    if i >= 1:
                        main_path(i - 1)

    _split_multi_waits(nc)
    return nc


def _prep_core_inputs(node_pad, neib_bf, W1, W1b, M2, W_node, W_neib, st=ST_FULL):
    """Split padded full arrays into per-core input dicts."""
    ncn = st * NODES_ST
    ins = []
    for c in range(NCORES):
        ins.append(
            {
                "node": np.ascontiguousarray(node_pad[c * ncn : (c + 1) * ncn]),
                "neib": np.ascontiguousarray(
                    neib_bf[c * ncn * K : (c + 1) * ncn * K].reshape(st, CH, RP, D)
                ),
                "w1f": W1,
                "w1b": W1b,
                "m2": M2,
                "wnode": W_node,
                "wneib": W_neib,
                "sel4": _sel4_const(),
            }
        )
    return ins


def kernel(node_feats, neib_feats, node_ids, neib_ids, W_att1, W_att2, W_node, W_neib):
    from concourse.bass_utils import run_bass_kernel_spmd

    node_feats = np.asarray(node_feats, dtype=np.float32)
    neib_feats = np.asarray(neib_feats, dtype=np.float32)
    W1 = np.ascontiguousarray(np.asarray(W_att1, dtype=np.float32))
    W2 = np.asarray(W_att2, dtype=np.float32)
    W_node = np.ascontiguousarray(np.asarray(W_node, dtype=np.float32))
    W_neib = np.ascontiguousarray(np.asarray(W_neib, dtype=np.float32))
    M2 = (W2.astype(np.float64) @ W2.astype(np.float64).T).astype(np.float32)
    W1b = W1.astype(ml_dtypes.bfloat16)

    n = node_feats.shape[0]
    node_pad = np.zeros((NPAD, D), dtype=np.float32)
    node_pad[:n] = node_feats
    neib_bf = np.zeros((NPAD * K, D), dtype=ml_dtypes.bfloat16)
    neib_bf[: n * K] = neib_feats.astype(ml_dtypes.bfloat16)

    if "nc" not in _module_cache:
        _module_cache["nc"] = build_module(ST_FULL)
    nc = _module_cache["nc"]

    in_maps = _prep_core_inputs(node_pad, neib_bf, W1, W1b, M2, W_node, W_neib)
    res = run_bass_kernel_spmd(nc, in_maps, core_ids=list(range(NCORES)))
    outs = np.concatenate([res.results[c]["out"] for c in range(NCORES)], axis=0)
    return np.ascontiguousarray(outs[:n])

